# revision 21
# baseline (speedup 1.0000x reference)
"""Bass/Trainium2 kernel for LightweightHypersphericalAttention.

Sharding: 8 cores = (batch b in 0..3) x (query half in 0..1).
Each core gets x_sh [1024, 512] (its query rows), ctx [2048, 512] (full
context for its batch), the weights, and radius; computes its [1024, 512]
slice of the final output. No collectives; host concatenates slices.

Structure (software-pipelined over heads: P1(0), P1(1), [flash(0),
P1(2)], [flash(1), P1(3)], flash(2), flash(3), oproj):
  P0  DMA x/ctx/W row-major, PE-transpose into chunk-major c-on-partition
      layouts (2 blocks per 2-bank psum tile), cast (optionally to fp8e4,
      W scaled x16 -- the scale cancels in the normalization).
  P1h project qT_raw/kT_raw (fp8 DoubleRow or bf16); sum-of-squares via
      all-ones-stationary matmuls that column-sum AND broadcast across
      partitions in one shot; factors via ACT ln/exp (exp(-.5 ln ss +
      ln r) = r/||q||) -- ln and exp share one ACT table, so no
      activation-table swaps anywhere in the kernel.
  P2h flash loop over 16 key chunks: S^T for both query halves into one
      2-bank psum tile; ONE exp [128,1024] per chunk (per-partition fk
      scale); AV accumulates in PSUM; denominator tiles accumulate on
      DVE; tail: all-ones matmul (colsum+broadcast), approx-reciprocal,
      multiply.
  P3  output projection.
"""

import numpy as np

P = 128
B, N, M, C, H = 4, 2048, 2048, 512, 4
D_V = 128
D_QK = 256
SCALE = float(D_V) ** -0.5
N_CORE = 1024          # query rows per core
NN = N_CORE // P       # 8 query chunks
MM = M // P            # 16 key chunks
CCH = C // P           # 4 channel chunks
NT = N_CORE // 512     # 2 query 512-tiles
MT = M // 512          # 4 key 512-tiles
W_SCALE = 16.0         # fp8 range helper; cancels in normalization

_NC_CACHE = {}


def _build(fp8: bool):
    import math
    import concourse.bass as bass
    import concourse.mybir as mybir
    import concourse.tile as tile
    from concourse import bacc
    from concourse.masks import make_identity
    from contextlib import ExitStack

    f32 = mybir.dt.float32
    bf16 = mybir.dt.bfloat16
    f8 = mybir.dt.float8e4
    mdt = f8 if fp8 else bf16
    DR = mybir.MatmulPerfMode.DoubleRow if fp8 else None
    wmul = W_SCALE if fp8 else 1.0
    Exp = mybir.ActivationFunctionType.Exp
    Ln = mybir.ActivationFunctionType.Ln
    MULT = mybir.AluOpType.mult
    ADD = mybir.AluOpType.add

    nc = bacc.Bacc(None, target_bir_lowering=False, debug=False)
    x_t = nc.dram_tensor("x_sh", [N_CORE, C], f32, kind="ExternalInput")
    c_t = nc.dram_tensor("ctx", [M, C], f32, kind="ExternalInput")
    wq_t = nc.dram_tensor("w_qkv", [2 * C, C], f32, kind="ExternalInput")
    wp_t = nc.dram_tensor("w_proj", [C, C], f32, kind="ExternalInput")
    rad_t = nc.dram_tensor("radius", [H], f32, kind="ExternalInput")
    out_t = nc.dram_tensor("out_sh", [N_CORE, C], f32, kind="ExternalOutput")

    with tile.TileContext(nc) as tc, ExitStack() as es:
        const = es.enter_context(tc.tile_pool(name="const", bufs=1))
        stage = es.enter_context(tc.tile_pool(name="stage", bufs=2))
        wts = es.enter_context(tc.tile_pool(name="wts", bufs=1))
        acts = es.enter_context(tc.tile_pool(name="acts", bufs=1))
        qk = es.enter_context(tc.tile_pool(name="qk", bufs=1))
        qbp = es.enter_context(tc.tile_pool(name="qbp", bufs=2))
        vp = es.enter_context(tc.tile_pool(name="vp", bufs=4))
        sqp = es.enter_context(tc.tile_pool(name="sqp", bufs=3))
        rows = es.enter_context(tc.tile_pool(name="rows", bufs=3))
        ptp = es.enter_context(tc.tile_pool(name="ptp", bufs=3))
        accp = es.enter_context(tc.tile_pool(name="accp", bufs=2))
        ocp = es.enter_context(tc.tile_pool(name="ocp", bufs=1))
        osb = es.enter_context(tc.tile_pool(name="osb", bufs=3))
        # PSUM: pair [128,1024] x2 (4 banks) + av x2 (2) + aux x2 (2) = 8
        ps_pr = es.enter_context(tc.tile_pool(name="ps_pr", bufs=2,
                                              space="PSUM"))
        ps_av = es.enter_context(tc.tile_pool(name="ps_av", bufs=2,
                                              space="PSUM"))
        ps_ax = es.enter_context(tc.tile_pool(name="ps_ax", bufs=2,
                                              space="PSUM"))

        # ---- constants ----
        identity = const.tile([P, P], f32)
        make_identity(nc, identity)
        allones = const.tile([P, P], bf16)
        nc.vector.memset(allones, 1.0)
        # D_all[:, j, i] = 1 if i == j else 0 -- selector that drops column
        # sums of moving tile j into psum row j (for the fk row layout).
        D_all = const.tile([P, MT, MT], bf16)
        nc.vector.memset(D_all, 0.0)
        for j in range(MT):
            nc.vector.memset(D_all[:, j, j:j + 1], 1.0)
        rad_b = const.tile([P, H], f32)
        rad_ap = rad_t[:]
        nc.sync.dma_start(out=rad_b, in_=bass.AP(
            tensor=rad_ap.tensor, offset=rad_ap.offset,
            ap=[[0, P], rad_ap.ap[0]]))
        # ln(r) and ln(r*SCALE): biases for the exp(-0.5*ln(ss) + bias)
        # rsqrt factors (r/||q|| and r*SCALE/||k_raw||).
        lnr = const.tile([P, H], f32)
        nc.scalar.activation(lnr, rad_b, Ln)
        rks = const.tile([P, H], f32)
        nc.scalar.mul(rks, rad_b, SCALE)
        lnrs = const.tile([P, H], f32)
        nc.scalar.activation(lnrs, rks, Ln)

        # ---- v tiles for all heads (issued early; consumed in P2) ----
        v_aug = []
        for h in range(H):
            vt = vp.tile([P, MM, D_V], bf16, tag="v", name=f"v_{h}")
            nc.gpsimd.dma_start(out=vt, in_=c_t[:, h * D_V:(h + 1) * D_V]
                                .rearrange("(j p) dv -> p j dv", p=P))
            v_aug.append(vt)

        # ---- P0: staged loads + transposes + casts ----
        WT = wts.tile([P, CCH, 2 * C], mdt, tag="WT")
        WpT = wts.tile([P, CCH, C], bf16, tag="WpT")
        xT = acts.tile([P, CCH, N_CORE], mdt, tag="xT")
        cT = acts.tile([P, CCH, M], mdt, tag="cT")
        qT8 = qk.tile([P, H, 2, N_CORE], mdt, tag="qT8")
        kT8 = qk.tile([P, H, 2, M], mdt, tag="kT8")

        cast_eng = [nc.scalar, nc.vector]
        dma_eng = [nc.sync, nc.scalar]
        dma_ct = [0]

        def ldblocks(src_ap, nblk, dest, d0, scale):
            stg = stage.tile([P, nblk, C], f32, tag="stg")
            dma_ct[0] += 1
            dma_eng[dma_ct[0] % 2].dma_start(out=stg, in_=src_ap)
            for bp in range(nblk // 2):       # 2 blocks per 2-bank tile
                pst = ps_pr.tile([P, 2, C], f32, tag="pair")
                for half in range(2):
                    blk = bp * 2 + half
                    for cc in range(CCH):
                        nc.tensor.transpose(
                            pst[:, half, cc * P:(cc + 1) * P],
                            stg[:, blk, cc * P:(cc + 1) * P], identity)
                # dest[:, cc, (d0+2bp+half)*P + q] <- pst[:, half, cc, q]
                dst = dest[:, :, (d0 + 2 * bp) * P:(d0 + 2 * bp + 2) * P]
                dst = dst.rearrange("p cc (half q) -> p half cc q", q=P)
                pv = pst.rearrange("p half (cc q) -> p half cc q", q=P)
                eng = cast_eng[bp % 2]
                if scale == 1.0:
                    if eng is nc.scalar:
                        nc.scalar.copy(out=dst, in_=pv)
                    else:
                        nc.vector.tensor_copy(out=dst, in_=pv)
                elif eng is nc.scalar:
                    nc.scalar.mul(dst, pv, scale)
                else:
                    nc.vector.tensor_scalar_mul(dst, pv, scale)

        x_blocks = x_t[:].rearrange("(nn p) c -> p nn c", p=P)
        ctx_blocks = c_t[:].rearrange("(nn p) c -> p nn c", p=P)
        wq_blocks = wq_t[:].rearrange("(oo p) c -> p oo c", p=P)
        ldblocks(x_blocks[:, :4, :], 4, xT, 0, 1.0)
        ldblocks(x_blocks[:, 4:, :], 4, xT, 4, 1.0)
        ldblocks(wq_blocks[:, :4, :], 4, WT, 0, wmul)
        for cb in range(4):
            ldblocks(ctx_blocks[:, 4 * cb:4 * cb + 4, :], 4, cT, 4 * cb, 1.0)
        ldblocks(wq_blocks[:, 4:, :], 4, WT, 4, wmul)
        ldblocks(wp_t[:].rearrange("(oo p) c -> p oo c", p=P), CCH, WpT, 0,
                 1.0)

        outcatT = ocp.tile([P, H, N_CORE], bf16, tag="ocT")

        def proj_mms(ps, o0, mov_t, c0, c1):
            """Accumulate [c0*512:(c0+1)*512 | c1...] cols of mov into ps."""
            for half, t in enumerate((c0, c1)):
                if fp8:
                    for ccp in range(2):
                        nc.tensor.matmul(
                            ps[:, half, :],
                            WT[:, 2 * ccp:2 * ccp + 2, o0:o0 + P],
                            mov_t[:, 2 * ccp:2 * ccp + 2,
                                  t * 512:(t + 1) * 512],
                            start=(ccp == 0), stop=(ccp == 1), perf_mode=DR)
                else:
                    for cc in range(CCH):
                        nc.tensor.matmul(
                            ps[:, half, :], WT[:, cc, o0:o0 + P],
                            mov_t[:, cc, t * 512:(t + 1) * 512],
                            start=(cc == 0), stop=(cc == CCH - 1))

        def p1(h):
            qTb = qbp.tile([P, 2, N_CORE], bf16, tag="qTb", name=f"qTb_{h}")
            for dc in range(2):
                o0 = h * D_QK + dc * P
                psq = ps_pr.tile([P, 2, 512], f32, tag="pair",
                                 name=f"pq{h}{dc}")
                proj_mms(psq, o0, xT, 0, 1)
                nc.vector.tensor_copy(out=qTb[:, dc, :]
                                      .rearrange("p (t n) -> p t n", n=512),
                                      in_=psq)
                psk0 = ps_pr.tile([P, 2, 512], f32, tag="pair",
                                  name=f"pk0{h}{dc}")
                proj_mms(psk0, o0, cT, 0, 1)
                nc.scalar.copy(out=kT8[:, h, dc, :1024]
                               .rearrange("p (t n) -> p t n", n=512),
                               in_=psk0)
                psk1 = ps_pr.tile([P, 2, 512], f32, tag="pair",
                                  name=f"pk1{h}{dc}")
                proj_mms(psk1, o0, cT, 2, 3)
                nc.vector.tensor_copy(out=kT8[:, h, dc, 1024:]
                                      .rearrange("p (t n) -> p t n", n=512),
                                      in_=psk1)

            # ---- fq: ss colsum+broadcast matmuls ----
            ssb_ps = ps_pr.tile([P, 2, 512], f32, tag="pair",
                                name=f"ssb_{h}")
            for nt in range(NT):
                sqt = sqp.tile([P, 2, 512], bf16, tag="sq")
                nc.vector.tensor_mul(sqt, qTb[:, :, nt * 512:(nt + 1) * 512],
                                     qTb[:, :, nt * 512:(nt + 1) * 512])
                for dc in range(2):
                    nc.tensor.matmul(ssb_ps[:, nt, :], allones,
                                     sqt[:, dc, :],
                                     start=(dc == 0), stop=(dc == 1))
            ssb = rows.tile([P, 2, 512], f32, tag="ssb", name=f"ssbs_{h}")
            nc.vector.tensor_copy(out=ssb, in_=ssb_ps)
            # ---- fk: rows via selector matmuls -> cols via transpose ----
            ssk = ps_ax.tile([P, 512], f32, tag="aux", name=f"ssk_{h}")
            for mt in range(MT):
                sqt = sqp.tile([P, 2, 512], bf16, tag="sq")
                nc.vector.tensor_mul(
                    sqt, kT8[:, h, :, mt * 512:(mt + 1) * 512],
                    kT8[:, h, :, mt * 512:(mt + 1) * 512])
                for dc in range(2):
                    nc.tensor.matmul(ssk[:MT, :], D_all[:, mt, :],
                                     sqt[:, dc, :],
                                     start=(mt == 0 and dc == 0),
                                     stop=(mt == MT - 1 and dc == 1))
            skrow = rows.tile([MT, 512], f32, tag="skrow", name=f"skr_{h}")
            nc.vector.tensor_copy(out=skrow, in_=ssk[:MT, :])
            sskc = rows.tile([P, 16], f32, tag="sskc", name=f"sskc_{h}")
            pst = ps_ax.tile([P, 512], f32, tag="aux", name=f"fkt_{h}")
            for b in range(4):
                nc.tensor.transpose(pst[:, b * MT:(b + 1) * MT],
                                    skrow[:, b * P:(b + 1) * P],
                                    identity[:MT, :MT])
            nc.vector.tensor_copy(out=sskc, in_=pst[:, :16])
            return qTb, ssb, sskc

        def p1_factors(h, qTb, ssb, sskc):
            """ln/exp factor chains; batched so ACT stays on one table."""
            fkc = rows.tile([P, MT, 4], f32, tag="fkc", name=f"fkc_{h}")
            lns = rows.tile([P, 2, 512], f32, tag="lns", name=f"lns_{h}")
            nc.scalar.activation(lns, ssb, Ln)
            lnk = rows.tile([P, 16], f32, tag="lnk", name=f"lnk_{h}")
            nc.scalar.activation(lnk, sskc, Ln)
            fqb = rows.tile([P, 2, 512], bf16, tag="fqb", name=f"fqb_{h}")
            nc.scalar.activation(fqb, lns, Exp, scale=-0.5,
                                 bias=lnr[:, h:h + 1])
            nc.scalar.activation(fkc.rearrange("p mt b -> p b mt"), lnk,
                                 Exp, scale=-0.5, bias=lnrs[:, h:h + 1])
            # qhat = qTb * fq[n] (broadcast tile), cast to matmul dtype
            nc.vector.tensor_tensor(
                qT8[:, h, :, :].rearrange("p dc (t n) -> p dc t n", n=512),
                qTb.rearrange("p dc (t n) -> p dc t n", n=512),
                fqb[:, None, :, :].to_broadcast((P, 2, 2, 512)),
                MULT)
            return fkc

        def flash(h, fkc):
            avos = [ps_av.tile([P, 512], f32, tag="av", name=f"avo_{h}_{nt}")
                    for nt in range(NT)]
            acc = accp.tile([P, 2, 512], bf16, tag="acc", name=f"acc_{h}")
            for j in range(MM):
                psS = ps_pr.tile([P, 2, 512], f32, tag="pair",
                                 name=f"s{h}_{j}")
                if fp8:
                    for nt in range(NT):
                        nc.tensor.matmul(
                            psS[:, nt, :], kT8[:, h, :, j * P:(j + 1) * P],
                            qT8[:, h, :, nt * 512:(nt + 1) * 512],
                            perf_mode=DR)
                else:
                    for dc in range(2):
                        for nt in range(NT):
                            nc.tensor.matmul(
                                psS[:, nt, :],
                                kT8[:, h, dc, j * P:(j + 1) * P],
                                qT8[:, h, dc, nt * 512:(nt + 1) * 512],
                                start=(dc == 0), stop=(dc == 1),
                                skip_group_check=True)
                PT = ptp.tile([P, 2, 512], bf16, tag="pt")
                nc.scalar.activation(PT, psS, Exp,
                                     scale=fkc[:, j // 4, j % 4:j % 4 + 1])
                for nt in range(NT):
                    nc.tensor.matmul(avos[nt], v_aug[h][:, j, :],
                                     PT[:, nt, :],
                                     start=(j == 0), stop=(j == MM - 1))
                if j == 0:
                    nc.vector.tensor_copy(out=acc, in_=PT)
                else:
                    nc.vector.tensor_tensor(acc, acc, PT, ADD)
            for nt in range(NT):
                denb = ps_ax.tile([P, 512], f32, tag="aux",
                                  name=f"dn_{h}_{nt}")
                nc.tensor.matmul(denb, allones, acc[:, nt, :])
                invb = rows.tile([P, 512], f32, tag="invb",
                                 name=f"invb_{h}_{nt}")
                nc.vector.reciprocal_approx_fast(out=invb, in_=denb)
                nc.vector.tensor_tensor(
                    outcatT[:, h, nt * 512:(nt + 1) * 512],
                    avos[nt], invb, MULT)

        # ---- phase-global schedule: all projections, batched factor
        # chains (single ACT table swap), then all flash loops ----
        parts = [p1(h) for h in range(H)]
        with tc.high_priority():
            fkcs = [p1_factors(h, *parts[h]) for h in range(H)]
        for h in range(H):
            flash(h, fkcs[h])

        # ---- P3: output projection ----
        out_engines = [nc.sync, nc.gpsimd, nc.scalar]
        for nn in range(NN):
            pso = ps_ax.tile([P, C], f32, tag="aux", name=f"po_{nn}")
            for cc in range(CCH):
                nc.tensor.matmul(pso, outcatT[:, cc, nn * P:(nn + 1) * P],
                                 WpT[:, cc, :],
                                 start=(cc == 0), stop=(cc == CCH - 1))
            o_sb = osb.tile([P, C], f32, tag="osb")
            nc.scalar.copy(out=o_sb, in_=pso)
            out_engines[nn % 3].dma_start(
                out=out_t[nn * P:(nn + 1) * P, :], in_=o_sb)

    nc.compile()
    return nc


def _get_nc(fp8: bool):
    if fp8 not in _NC_CACHE:
        _NC_CACHE[fp8] = _build(fp8)
    return _NC_CACHE[fp8]


def kernel(x, context, W_qkv, W_proj, radius, _trace=False, _bf16=True,
           _fp8=False):
    # fp8e4m3 q/k paths measure ~2-4e-2 rel err (over the 2e-2 gate), so
    # the bf16 build is the default regardless of _bf16.
    from concourse.bass_utils import run_bass_kernel_spmd

    x = np.ascontiguousarray(np.asarray(x, dtype=np.float32))
    context = np.ascontiguousarray(np.asarray(context, dtype=np.float32))
    W_qkv = np.ascontiguousarray(np.asarray(W_qkv, dtype=np.float32))
    W_proj = np.ascontiguousarray(np.asarray(W_proj, dtype=np.float32))
    radius = np.ascontiguousarray(np.asarray(radius, dtype=np.float32))

    nc = _get_nc(bool(_fp8))
    in_maps = []
    for i in range(8):
        b, half = i // 2, i % 2
        in_maps.append({
            "x_sh": x[b, half * N_CORE:(half + 1) * N_CORE, :],
            "ctx": context[b],
            "w_qkv": W_qkv,
            "w_proj": W_proj,
            "radius": radius,
        })
    res = run_bass_kernel_spmd(nc, in_maps, list(range(8)), trace=_trace)
    out = np.empty((B, N, C), dtype=np.float32)
    for i in range(8):
        b, half = i // 2, i % 2
        out[b, half * N_CORE:(half + 1) * N_CORE, :] = res.results[i]["out_sh"]
    if _trace:
        return out, res
    return out


# revision 22
# speedup vs baseline: 1.1672x; 1.1672x over previous
"""Bass/Trainium2 kernel for LightweightHypersphericalAttention.

Sharding: 8 cores = (batch b in 0..3) x (query half in 0..1).
Each core gets x_sh [1024, 512] (its query rows), ctx [2048, 512] (full
context for its batch), the weights, and radius; computes its [1024, 512]
slice of the final output. No collectives; host concatenates slices.

Structure (software-pipelined over heads: P1(0), P1(1), [flash(0),
P1(2)], [flash(1), P1(3)], flash(2), flash(3), oproj):
  P0  DMA x/ctx/W row-major, PE-transpose into chunk-major c-on-partition
      layouts (2 blocks per 2-bank psum tile), cast (optionally to fp8e4,
      W scaled x16 -- the scale cancels in the normalization).
  P1h project qT_raw/kT_raw (fp8 DoubleRow or bf16); sum-of-squares via
      all-ones-stationary matmuls that column-sum AND broadcast across
      partitions in one shot; factors via ACT ln/exp (exp(-.5 ln ss +
      ln r) = r/||q||) -- ln and exp share one ACT table, so no
      activation-table swaps anywhere in the kernel.
  P2h flash loop over 16 key chunks: S^T for both query halves into one
      2-bank psum tile; ONE exp [128,1024] per chunk (per-partition fk
      scale); AV accumulates in PSUM; denominator tiles accumulate on
      DVE; tail: all-ones matmul (colsum+broadcast), approx-reciprocal,
      multiply.
  P3  output projection.
"""

import numpy as np

P = 128
B, N, M, C, H = 4, 2048, 2048, 512, 4
D_V = 128
D_QK = 256
SCALE = float(D_V) ** -0.5
N_CORE = 1024          # query rows per core
NN = N_CORE // P       # 8 query chunks
MM = M // P            # 16 key chunks
CCH = C // P           # 4 channel chunks
NT = N_CORE // 512     # 2 query 512-tiles
MT = M // 512          # 4 key 512-tiles
W_SCALE = 16.0         # fp8 range helper; cancels in normalization

_NC_CACHE = {}


def _build(fp8: bool):
    import math
    import concourse.bass as bass
    import concourse.mybir as mybir
    import concourse.tile as tile
    from concourse import bacc
    from concourse.masks import make_identity
    from contextlib import ExitStack

    f32 = mybir.dt.float32
    bf16 = mybir.dt.bfloat16
    f8 = mybir.dt.float8e4
    mdt = f8 if fp8 else bf16
    DR = mybir.MatmulPerfMode.DoubleRow if fp8 else None
    wmul = W_SCALE if fp8 else 1.0
    Exp = mybir.ActivationFunctionType.Exp
    Ln = mybir.ActivationFunctionType.Ln
    MULT = mybir.AluOpType.mult
    ADD = mybir.AluOpType.add

    nc = bacc.Bacc(None, target_bir_lowering=False, debug=False)
    x_t = nc.dram_tensor("x_sh", [N_CORE, C], f32, kind="ExternalInput")
    c_t = nc.dram_tensor("ctx", [M, C], f32, kind="ExternalInput")
    wq_t = nc.dram_tensor("w_qkv", [2 * C, C], f32, kind="ExternalInput")
    wp_t = nc.dram_tensor("w_proj", [C, C], f32, kind="ExternalInput")
    rad_t = nc.dram_tensor("radius", [H], f32, kind="ExternalInput")
    out_t = nc.dram_tensor("out_sh", [N_CORE, C], f32, kind="ExternalOutput")

    with tile.TileContext(nc) as tc, ExitStack() as es:
        const = es.enter_context(tc.tile_pool(name="const", bufs=1))
        stage = es.enter_context(tc.tile_pool(name="stage", bufs=2))
        wts = es.enter_context(tc.tile_pool(name="wts", bufs=1))
        acts = es.enter_context(tc.tile_pool(name="acts", bufs=1))
        qk = es.enter_context(tc.tile_pool(name="qk", bufs=1))
        qbp = es.enter_context(tc.tile_pool(name="qbp", bufs=2))
        vp = es.enter_context(tc.tile_pool(name="vp", bufs=4))
        sqp = es.enter_context(tc.tile_pool(name="sqp", bufs=3))
        rows = es.enter_context(tc.tile_pool(name="rows", bufs=3))
        ptp = es.enter_context(tc.tile_pool(name="ptp", bufs=3))
        accp = es.enter_context(tc.tile_pool(name="accp", bufs=2))
        ocp = es.enter_context(tc.tile_pool(name="ocp", bufs=1))
        osb = es.enter_context(tc.tile_pool(name="osb", bufs=3))
        # PSUM: pair [128,1024] x2 (4 banks) + av x2 (2) + aux x2 (2) = 8
        ps_pr = es.enter_context(tc.tile_pool(name="ps_pr", bufs=2,
                                              space="PSUM"))
        ps_av = es.enter_context(tc.tile_pool(name="ps_av", bufs=2,
                                              space="PSUM"))
        ps_ax = es.enter_context(tc.tile_pool(name="ps_ax", bufs=2,
                                              space="PSUM"))

        # ---- constants ----
        identity = const.tile([P, P], f32)
        make_identity(nc, identity)
        allones = const.tile([P, P], bf16)
        nc.vector.memset(allones, 1.0)
        # D_all[:, j, i] = 1 if i == j else 0 -- selector that drops column
        # sums of moving tile j into psum row j (for the fk row layout).
        D_all = const.tile([P, MT, MT], bf16)
        nc.vector.memset(D_all, 0.0)
        for j in range(MT):
            nc.vector.memset(D_all[:, j, j:j + 1], 1.0)
        rad_b = const.tile([P, H], f32)
        rad_ap = rad_t[:]
        nc.sync.dma_start(out=rad_b, in_=bass.AP(
            tensor=rad_ap.tensor, offset=rad_ap.offset,
            ap=[[0, P], rad_ap.ap[0]]))
        # ln(r) and ln(r*SCALE): biases for the exp(-0.5*ln(ss) + bias)
        # rsqrt factors (r/||q|| and r*SCALE/||k_raw||).
        lnr = const.tile([P, H], f32)
        nc.scalar.activation(lnr, rad_b, Ln)
        rks = const.tile([P, H], f32)
        nc.scalar.mul(rks, rad_b, SCALE)
        lnrs = const.tile([P, H], f32)
        nc.scalar.activation(lnrs, rks, Ln)

        # ---- v tiles for all heads (issued early; consumed in P2) ----
        v_aug = []
        for h in range(H):
            vt = vp.tile([P, MM, D_V], bf16, tag="v", name=f"v_{h}")
            nc.gpsimd.dma_start(out=vt, in_=c_t[:, h * D_V:(h + 1) * D_V]
                                .rearrange("(j p) dv -> p j dv", p=P))
            v_aug.append(vt)

        # ---- P0: staged loads + transposes + casts ----
        WT = wts.tile([P, CCH, 2 * C], mdt, tag="WT")
        WpT = wts.tile([P, CCH, C], bf16, tag="WpT")
        xT = acts.tile([P, CCH, N_CORE], mdt, tag="xT")
        cT = acts.tile([P, CCH, M], mdt, tag="cT")
        qT8 = qk.tile([P, H, 2, N_CORE], mdt, tag="qT8")
        kT8 = qk.tile([P, H, 2, M], mdt, tag="kT8")

        cast_eng = [nc.scalar, nc.vector]
        dma_eng = [nc.sync, nc.scalar]
        dma_ct = [0]

        def ldblocks(src_ap, nblk, dest, d0, scale):
            stg = stage.tile([P, nblk, C], f32, tag="stg")
            dma_ct[0] += 1
            dma_eng[dma_ct[0] % 2].dma_start(out=stg, in_=src_ap)
            for bp in range(nblk // 2):       # 2 blocks per 2-bank tile
                pst = ps_pr.tile([P, 2, C], f32, tag="pair")
                for half in range(2):
                    blk = bp * 2 + half
                    for cc in range(CCH):
                        nc.tensor.transpose(
                            pst[:, half, cc * P:(cc + 1) * P],
                            stg[:, blk, cc * P:(cc + 1) * P], identity)
                # dest[:, cc, (d0+2bp+half)*P + q] <- pst[:, half, cc, q]
                dst = dest[:, :, (d0 + 2 * bp) * P:(d0 + 2 * bp + 2) * P]
                dst = dst.rearrange("p cc (half q) -> p half cc q", q=P)
                pv = pst.rearrange("p half (cc q) -> p half cc q", q=P)
                eng = cast_eng[bp % 2]
                if scale == 1.0:
                    if eng is nc.scalar:
                        nc.scalar.copy(out=dst, in_=pv)
                    else:
                        nc.vector.tensor_copy(out=dst, in_=pv)
                elif eng is nc.scalar:
                    nc.scalar.mul(dst, pv, scale)
                else:
                    nc.vector.tensor_scalar_mul(dst, pv, scale)

        x_blocks = x_t[:].rearrange("(nn p) c -> p nn c", p=P)
        ctx_blocks = c_t[:].rearrange("(nn p) c -> p nn c", p=P)
        wq_blocks = wq_t[:].rearrange("(oo p) c -> p oo c", p=P)
        ldblocks(x_blocks[:, :4, :], 4, xT, 0, 1.0)
        ldblocks(x_blocks[:, 4:, :], 4, xT, 4, 1.0)
        ldblocks(wq_blocks[:, :4, :], 4, WT, 0, wmul)
        for cb in range(4):
            ldblocks(ctx_blocks[:, 4 * cb:4 * cb + 4, :], 4, cT, 4 * cb, 1.0)
        ldblocks(wq_blocks[:, 4:, :], 4, WT, 4, wmul)
        ldblocks(wp_t[:].rearrange("(oo p) c -> p oo c", p=P), CCH, WpT, 0,
                 1.0)

        outcatT = ocp.tile([P, H, N_CORE], bf16, tag="ocT")

        def proj_mms(ps, o0, mov_t, c0, c1):
            """Accumulate [c0*512:(c0+1)*512 | c1...] cols of mov into ps."""
            for half, t in enumerate((c0, c1)):
                if fp8:
                    for ccp in range(2):
                        nc.tensor.matmul(
                            ps[:, half, :],
                            WT[:, 2 * ccp:2 * ccp + 2, o0:o0 + P],
                            mov_t[:, 2 * ccp:2 * ccp + 2,
                                  t * 512:(t + 1) * 512],
                            start=(ccp == 0), stop=(ccp == 1), perf_mode=DR)
                else:
                    for cc in range(CCH):
                        nc.tensor.matmul(
                            ps[:, half, :], WT[:, cc, o0:o0 + P],
                            mov_t[:, cc, t * 512:(t + 1) * 512],
                            start=(cc == 0), stop=(cc == CCH - 1))

        def p1(h):
            qTb = qbp.tile([P, 2, N_CORE], bf16, tag="qTb", name=f"qTb_{h}")
            for dc in range(2):
                o0 = h * D_QK + dc * P
                psq = ps_pr.tile([P, 2, 512], f32, tag="pair",
                                 name=f"pq{h}{dc}")
                proj_mms(psq, o0, xT, 0, 1)
                nc.vector.tensor_copy(out=qTb[:, dc, :]
                                      .rearrange("p (t n) -> p t n", n=512),
                                      in_=psq)
                psk0 = ps_pr.tile([P, 2, 512], f32, tag="pair",
                                  name=f"pk0{h}{dc}")
                proj_mms(psk0, o0, cT, 0, 1)
                nc.scalar.copy(out=kT8[:, h, dc, :1024]
                               .rearrange("p (t n) -> p t n", n=512),
                               in_=psk0)
                psk1 = ps_pr.tile([P, 2, 512], f32, tag="pair",
                                  name=f"pk1{h}{dc}")
                proj_mms(psk1, o0, cT, 2, 3)
                nc.vector.tensor_copy(out=kT8[:, h, dc, 1024:]
                                      .rearrange("p (t n) -> p t n", n=512),
                                      in_=psk1)

            # ---- fq: ss colsum+broadcast matmuls ----
            ssb_ps = ps_pr.tile([P, 2, 512], f32, tag="pair",
                                name=f"ssb_{h}")
            for nt in range(NT):
                sqt = sqp.tile([P, 2, 512], bf16, tag="sq")
                nc.vector.tensor_mul(sqt, qTb[:, :, nt * 512:(nt + 1) * 512],
                                     qTb[:, :, nt * 512:(nt + 1) * 512])
                for dc in range(2):
                    nc.tensor.matmul(ssb_ps[:, nt, :], allones,
                                     sqt[:, dc, :],
                                     start=(dc == 0), stop=(dc == 1))
            ssb = rows.tile([P, 2, 512], f32, tag="ssb", name=f"ssbs_{h}")
            nc.vector.tensor_copy(out=ssb, in_=ssb_ps)
            # ---- fk: rows via selector matmuls -> cols via transpose ----
            ssk = ps_ax.tile([P, 512], f32, tag="aux", name=f"ssk_{h}")
            for mt in range(MT):
                sqt = sqp.tile([P, 2, 512], bf16, tag="sq")
                nc.vector.tensor_mul(
                    sqt, kT8[:, h, :, mt * 512:(mt + 1) * 512],
                    kT8[:, h, :, mt * 512:(mt + 1) * 512])
                for dc in range(2):
                    nc.tensor.matmul(ssk[:MT, :], D_all[:, mt, :],
                                     sqt[:, dc, :],
                                     start=(mt == 0 and dc == 0),
                                     stop=(mt == MT - 1 and dc == 1))
            skrow = rows.tile([MT, 512], f32, tag="skrow", name=f"skr_{h}")
            nc.vector.tensor_copy(out=skrow, in_=ssk[:MT, :])
            sskc = rows.tile([P, 16], f32, tag="sskc", name=f"sskc_{h}")
            pst = ps_ax.tile([P, 512], f32, tag="aux", name=f"fkt_{h}")
            for b in range(4):
                nc.tensor.transpose(pst[:, b * MT:(b + 1) * MT],
                                    skrow[:, b * P:(b + 1) * P],
                                    identity[:MT, :MT])
            nc.vector.tensor_copy(out=sskc, in_=pst[:, :16])
            return qTb, ssb, sskc

        def p1_factors(h, qTb, ssb, sskc):
            """ln/exp factor chains; batched so ACT stays on one table."""
            fkc = rows.tile([P, MT, 4], f32, tag="fkc", name=f"fkc_{h}")
            lns = rows.tile([P, 2, 512], f32, tag="lns", name=f"lns_{h}")
            nc.scalar.activation(lns, ssb, Ln)
            lnk = rows.tile([P, 16], f32, tag="lnk", name=f"lnk_{h}")
            nc.scalar.activation(lnk, sskc, Ln)
            fqb = rows.tile([P, 2, 512], bf16, tag="fqb", name=f"fqb_{h}")
            nc.scalar.activation(fqb, lns, Exp, scale=-0.5,
                                 bias=lnr[:, h:h + 1])
            nc.scalar.activation(fkc.rearrange("p mt b -> p b mt"), lnk,
                                 Exp, scale=-0.5, bias=lnrs[:, h:h + 1])
            # qhat = qTb * fq[n] (broadcast tile), cast to matmul dtype
            nc.vector.tensor_tensor(
                qT8[:, h, :, :].rearrange("p dc (t n) -> p dc t n", n=512),
                qTb.rearrange("p dc (t n) -> p dc t n", n=512),
                fqb[:, None, :, :].to_broadcast((P, 2, 2, 512)),
                MULT)
            return fkc

        def flash(h, fkc):
            avos = [ps_av.tile([P, 512], f32, tag="av", name=f"avo_{h}_{nt}")
                    for nt in range(NT)]
            acc = accp.tile([P, 2, 512], bf16, tag="acc", name=f"acc_{h}")
            for j in range(MM):
                psS = ps_pr.tile([P, 2, 512], f32, tag="pair",
                                 name=f"s{h}_{j}")
                if fp8:
                    for nt in range(NT):
                        nc.tensor.matmul(
                            psS[:, nt, :], kT8[:, h, :, j * P:(j + 1) * P],
                            qT8[:, h, :, nt * 512:(nt + 1) * 512],
                            perf_mode=DR)
                else:
                    for dc in range(2):
                        for nt in range(NT):
                            nc.tensor.matmul(
                                psS[:, nt, :],
                                kT8[:, h, dc, j * P:(j + 1) * P],
                                qT8[:, h, dc, nt * 512:(nt + 1) * 512],
                                start=(dc == 0), stop=(dc == 1),
                                skip_group_check=True)
                PT = ptp.tile([P, 2, 512], bf16, tag="pt")
                nc.scalar.activation(PT, psS, Exp,
                                     scale=fkc[:, j // 4, j % 4:j % 4 + 1])
                for nt in range(NT):
                    nc.tensor.matmul(avos[nt], v_aug[h][:, j, :],
                                     PT[:, nt, :],
                                     start=(j == 0), stop=(j == MM - 1))
                if j == 0:
                    nc.vector.tensor_copy(out=acc, in_=PT)
                else:
                    nc.vector.tensor_tensor(acc, acc, PT, ADD)
            for nt in range(NT):
                denb = ps_ax.tile([P, 512], f32, tag="aux",
                                  name=f"dn_{h}_{nt}")
                nc.tensor.matmul(denb, allones, acc[:, nt, :])
                invb = rows.tile([P, 512], f32, tag="invb",
                                 name=f"invb_{h}_{nt}")
                nc.vector.reciprocal_approx_fast(out=invb, in_=denb)
                nc.vector.tensor_tensor(
                    outcatT[:, h, nt * 512:(nt + 1) * 512],
                    avos[nt], invb, MULT)

        # ---- phase-global schedule: all projections, batched factor
        # chains (single ACT table swap), then all flash loops ----
        parts = [p1(h) for h in range(H)]
        fkcs = [p1_factors(h, *parts[h]) for h in range(H)]
        for h in range(H):
            flash(h, fkcs[h])

        # ---- P3: output projection ----
        out_engines = [nc.sync, nc.gpsimd, nc.scalar]
        for nn in range(NN):
            pso = ps_ax.tile([P, C], f32, tag="aux", name=f"po_{nn}")
            for cc in range(CCH):
                nc.tensor.matmul(pso, outcatT[:, cc, nn * P:(nn + 1) * P],
                                 WpT[:, cc, :],
                                 start=(cc == 0), stop=(cc == CCH - 1))
            o_sb = osb.tile([P, C], f32, tag="osb")
            nc.scalar.copy(out=o_sb, in_=pso)
            out_engines[nn % 3].dma_start(
                out=out_t[nn * P:(nn + 1) * P, :], in_=o_sb)

    nc.compile()
    return nc


def _get_nc(fp8: bool):
    if fp8 not in _NC_CACHE:
        _NC_CACHE[fp8] = _build(fp8)
    return _NC_CACHE[fp8]


def kernel(x, context, W_qkv, W_proj, radius, _trace=False, _bf16=True,
           _fp8=False):
    # fp8e4m3 q/k paths measure ~2-4e-2 rel err (over the 2e-2 gate), so
    # the bf16 build is the default regardless of _bf16.
    from concourse.bass_utils import run_bass_kernel_spmd

    x = np.ascontiguousarray(np.asarray(x, dtype=np.float32))
    context = np.ascontiguousarray(np.asarray(context, dtype=np.float32))
    W_qkv = np.ascontiguousarray(np.asarray(W_qkv, dtype=np.float32))
    W_proj = np.ascontiguousarray(np.asarray(W_proj, dtype=np.float32))
    radius = np.ascontiguousarray(np.asarray(radius, dtype=np.float32))

    nc = _get_nc(bool(_fp8))
    in_maps = []
    for i in range(8):
        b, half = i // 2, i % 2
        in_maps.append({
            "x_sh": x[b, half * N_CORE:(half + 1) * N_CORE, :],
            "ctx": context[b],
            "w_qkv": W_qkv,
            "w_proj": W_proj,
            "radius": radius,
        })
    res = run_bass_kernel_spmd(nc, in_maps, list(range(8)), trace=_trace)
    out = np.empty((B, N, C), dtype=np.float32)
    for i in range(8):
        b, half = i // 2, i % 2
        out[b, half * N_CORE:(half + 1) * N_CORE, :] = res.results[i]["out_sh"]
    if _trace:
        return out, res
    return out
